# revision 4
# baseline (speedup 1.0000x reference)
"""Multi-head masked self-attention on 8 trn2 NeuronCores.

Problem: B=2, T=2048, H=1024, nH=16 heads (head_dim=64), causal softmax
attention with QKV projections; scores scaled by 1/sqrt(H).

Sharding: heads across cores (2 heads per core), both batches on every
core. QKV weights column-sharded by head: core m gets rows
[128m, 128m+128) of each projection matrix.

Per-core device program (mixed precision, tolerance 2e-2):

  x^T [1024, 2048] streamed in bf16 -> SBUF [128, 8cb, 2048]
  Q^T/K^T [128(2h*64d), T] = Wn^T @ x^T in bf16 (PE, 8-block f32 PSUM
      accumulation); DVE evicts with per-partition bias add straight to
      fp8(e4m3) tiles laid out [128, 2, T] whose second contraction tile
      is persistent zeros (DoubleRow zero-padding).
  V^T likewise but evicted to bf16 per-head tiles vt_h [96, T] (head1
      via cross-partition DVE copy 64:128 -> 0:64); row 64 = ones.
      One XBAR dma_start_transpose per head builds V' [128k, 16kb, 96]
      with column 64 = ones (softmax denominator column).
  Scores (transposed): S^T[k, q] fp8 DoubleRow matmuls (0.5 cyc/row),
      contraction [64, 2, .] zero-padded, q chunks of <=256.
      ACT evicts exp(S/32) -> bf16 P tiles [128, 2, 512]; causal mask
      applied post-exp as gpsimd affine_select zeroing on the diagonal
      128-col block (both heads in one op).
  O'^T [65, 512] = sum_kb V'[:, kb, :65].T @ P^T (bf16 PE accumulation;
      row 64 accumulates Z). DVE evicts psO -> SBUF f32, DMA to DRAM
      out [B, 2, 65, T]. Host divides by Z and transposes (cheap).

  Emission is software-pipelined: batch b1's projections are emitted
  interleaved with b0's attention q-tiles (attention is ACT/exp-bound,
  projections are PE-bound), and b0's projections for the next rep
  overlap b1's attention.
"""
import sys

sys.path.insert(0, "/opt/trn_rl_repo")

import numpy as np
import ml_dtypes

B = 2
T = 2048
H = 1024
NHEADS = 16
HD = 64
NCORES = 8
P = 128
CB = H // P            # 8 contraction blocks for projections
QTILE = 512
NQT = T // QTILE       # 4 q-tiles
NKB = T // P           # 16 k-blocks
SCALE = 1.0 / np.sqrt(np.float32(H))  # 1/32
VROWS = 96             # XBAR-transposed V rows (64 d + ones@64 + pad)


def _build_program(reps: int = 1):
    import contextlib
    import concourse.tile as tile
    from concourse import bacc, mybir
    from concourse.bass import ts

    F32 = mybir.dt.float32
    BF16 = mybir.dt.bfloat16
    F8 = mybir.dt.float8e4
    ActF = mybir.ActivationFunctionType
    DR = mybir.MatmulPerfMode.DoubleRow

    nc = bacc.Bacc("TRN2", target_bir_lowering=False, debug=False)

    xt_d = nc.dram_tensor("xt", [B, H, T], BF16, kind="ExternalInput")
    w_d = {
        n: nc.dram_tensor(f"w{n}t", [H, P], BF16, kind="ExternalInput")
        for n in "qkv"
    }
    b_d = {
        n: nc.dram_tensor(f"b{n}", [P], F32, kind="ExternalInput")
        for n in "qkv"
    }
    out_d = nc.dram_tensor("out", [B, 2, HD + 1, T], F32,
                           kind="ExternalOutput")

    with tile.TileContext(nc) as tc:
        with (
            tc.tile_pool(name="const", bufs=1) as const,
            tc.tile_pool(name="xt", bufs=2) as xt_pool,
            tc.tile_pool(name="vp", bufs=2) as vp_pool,
            tc.tile_pool(name="pt", bufs=4) as pt_pool,
            tc.tile_pool(name="osb", bufs=4) as osb_pool,
            tc.tile_pool(name="psproj", bufs=2, space="PSUM") as psproj,
            tc.tile_pool(name="psS", bufs=2, space="PSUM") as psS_pool,
            tc.tile_pool(name="pso", bufs=1, space="PSUM") as pso,
        ):
            # ---- persistent tiles ----
            w_sb = {}
            bias_sb = {}
            for n in "qkv":
                w_sb[n] = const.tile([P, CB, P], BF16, tag=f"w{n}", name=f"w{n}")
                nc.sync.dma_start(
                    w_sb[n][:],
                    w_d[n][:].rearrange("(cb p) m -> p cb m", p=P),
                )
                bias_sb[n] = const.tile([P, 1], F32, tag=f"b{n}", name=f"b{n}")
                nc.sync.dma_start(bias_sb[n][:], b_d[n][:, None])

            # fp8 Q/K tiles, [128(2h*64d), 2, T]; [:, 1, :] stays zero
            # (DoubleRow zero-pad contraction tile)
            q8 = {}
            k8 = {}
            for b in range(B):
                q8[b] = const.tile([P, 2, T], F8, tag=f"q8_{b}", name=f"q8_{b}")
                k8[b] = const.tile([P, 2, T], F8, tag=f"k8_{b}", name=f"k8_{b}")
                nc.gpsimd.memset(q8[b][:, 1, :], 0.0)
                nc.gpsimd.memset(k8[b][:, 1, :], 0.0)

            # bf16 V^T staging per head: rows 0:64 = d, row 64 = ones,
            # rows 65:96 = zeros (never read past col 64 after transpose)
            vt = {}
            for b in range(B):
                for h in range(2):
                    t_ = const.tile([VROWS, T], BF16, tag=f"vt{h}_{b}", name=f"vt{h}_{b}")
                    nc.gpsimd.memset(t_[HD:VROWS, :], 0.0)
                    nc.vector.memset(t_[HD : HD + 1, :], 1.0)
                    vt[(h, b)] = t_

            xt_tiles = {}
            vp_tiles = {}

            def load_xt(b):
                xtile = xt_pool.tile([P, CB, T], BF16, tag="xt")
                for cb in range(CB):
                    nc.scalar.dma_start(xtile[:, cb, :], xt_d[b, ts(cb, P), :])
                xt_tiles[b] = xtile

            def proj_slice(b, tt):
                """QKV projections for T-slice tt of batch b."""
                xtile = xt_tiles[b]
                sl = slice(tt * QTILE, (tt + 1) * QTILE)
                for n in "qkv":
                    ps = psproj.tile([P, QTILE], F32, tag="mm")
                    for cb in range(CB):
                        nc.tensor.matmul(
                            ps[:],
                            w_sb[n][:, cb, :],
                            xtile[:, cb, sl],
                            start=(cb == 0),
                            stop=(cb == CB - 1),
                        )
                    if n == "v":
                        nc.vector.tensor_scalar_add(
                            vt[(0, b)][:HD, sl], ps[:HD, :], bias_sb[n][:HD]
                        )
                        nc.vector.tensor_scalar_add(
                            vt[(1, b)][:HD, sl], ps[HD:, :], bias_sb[n][HD:]
                        )
                    else:
                        dst = q8[b] if n == "q" else k8[b]
                        nc.vector.tensor_scalar_add(
                            dst[:, 0, sl], ps[:], bias_sb[n][:]
                        )

            def vprime_build(b):
                for h in range(2):
                    vp = vp_pool.tile([P, NKB, VROWS], BF16, tag=f"vp{h}")
                    nc.sync.dma_start_transpose(vp[:], vt[(h, b)][:])
                    vp_tiles[(h, b)] = vp

            def attn_qt(b, qt):
                nkb = 4 * qt + 4
                psO = [
                    pso.tile([HD + 1, QTILE], F32, tag=f"o{h}",
                             name=f"psO{h}")
                    for h in range(2)
                ]
                q0 = qt * QTILE
                for kb in range(nkb):
                    i = kb - 4 * qt
                    lo = max(i, 0) * P
                    psS = psS_pool.tile([P, 2, QTILE], F32, tag="s")
                    for h in range(2):
                        c0 = lo
                        while c0 < QTILE:
                            n_ = min(256, QTILE - c0)
                            nc.tensor.matmul(
                                psS[:, h, c0 : c0 + n_],
                                k8[b][ts(h, HD), :, ts(kb, P)],
                                q8[b][ts(h, HD), :, q0 + c0 : q0 + c0 + n_],
                                perf_mode=DR,
                            )
                            c0 += n_
                    pt = pt_pool.tile([P, 2, QTILE], BF16, tag="pt")
                    nc.scalar.activation(
                        pt[:, :, lo:],
                        psS[:, :, lo:],
                        ActF.Exp,
                        scale=float(SCALE),
                    )
                    if i >= 0:
                        # zero masked (k_local > q_local) on the diagonal
                        # 128-col block, both heads at once
                        nc.gpsimd.affine_select(
                            out=pt[:, :, lo : lo + P],
                            in_=pt[:, :, lo : lo + P],
                            compare_op=mybir.AluOpType.is_ge,
                            fill=0.0,
                            base=0,
                            pattern=[[0, 2], [1, P]],
                            channel_multiplier=-1,
                        )
                    for h in range(2):
                        nc.tensor.matmul(
                            psO[h][:, lo:],
                            vp_tiles[(h, b)][:, kb, : HD + 1],
                            pt[:, h, lo:],
                            start=(kb == 0),
                            stop=(kb == nkb - 1),
                        )
                for h in range(2):
                    oT = osb_pool.tile([HD + 1, QTILE], F32, tag="oT")
                    nc.vector.tensor_copy(oT[:], psO[h][:])
                    nc.sync.dma_start(out_d[b, h, :, ts(qt, QTILE)], oT[:])

            # ---- prologue: batch 0 projections ----
            load_xt(0)
            for tt in range(NQT):
                proj_slice(0, tt)
            vprime_build(0)

            rep_ctx = (
                tc.For_i(0, reps, 1,
                         hint_engines=(mybir.EngineType.PE,
                                       mybir.EngineType.Activation,
                                       mybir.EngineType.DVE,
                                       mybir.EngineType.Pool,
                                       mybir.EngineType.SP))
                if reps > 1 else contextlib.nullcontext()
            )
            with rep_ctx:
                load_xt(1)
                for qt in range(NQT):
                    attn_qt(0, qt)
                    proj_slice(1, qt)
                vprime_build(1)
                load_xt(0)
                for qt in range(NQT):
                    attn_qt(1, qt)
                    proj_slice(0, qt)
                vprime_build(0)

    nc.compile()
    return nc


def _make_in_maps(inputs):
    x = np.ascontiguousarray(np.asarray(inputs["x"], np.float32))
    xt = np.ascontiguousarray(x.transpose(0, 2, 1)).astype(
        ml_dtypes.bfloat16
    )
    Wq, Wk, Wv = inputs["Wq"], inputs["Wk"], inputs["Wv"]
    bq, bk, bv = inputs["bq"], inputs["bk"], inputs["bv"]

    in_maps = []
    for m in range(NCORES):
        sl = slice(m * P, (m + 1) * P)  # 128 output channels = 2 heads
        in_maps.append({
            "xt": xt,
            "wqt": np.ascontiguousarray(
                np.asarray(Wq, np.float32)[sl, :].T
            ).astype(ml_dtypes.bfloat16),
            "wkt": np.ascontiguousarray(
                np.asarray(Wk, np.float32)[sl, :].T
            ).astype(ml_dtypes.bfloat16),
            "wvt": np.ascontiguousarray(
                np.asarray(Wv, np.float32)[sl, :].T
            ).astype(ml_dtypes.bfloat16),
            "bq": np.ascontiguousarray(np.asarray(bq, np.float32)[sl]),
            "bk": np.ascontiguousarray(np.asarray(bk, np.float32)[sl]),
            "bv": np.ascontiguousarray(np.asarray(bv, np.float32)[sl]),
        })
    return in_maps


_CACHED = {}


def kernel(x, Wq, bq, Wk, bk, Wv, bv):
    from concourse.bass_utils import run_bass_kernel_spmd

    if "nc" not in _CACHED:
        _CACHED["nc"] = _build_program()
    nc = _CACHED["nc"]

    in_maps = _make_in_maps(dict(
        x=x, Wq=Wq, bq=bq, Wk=Wk, bk=bk, Wv=Wv, bv=bv,
    ))

    res = run_bass_kernel_spmd(nc, in_maps, core_ids=list(range(NCORES)))

    parts = []
    for m in range(NCORES):
        o = res.results[m]["out"]  # [B, 2, 65, T] f32
        num = o[:, :, :HD, :]
        z = o[:, :, HD : HD + 1, :]
        core_out = (num / z).transpose(0, 3, 1, 2).reshape(B, T, 2 * HD)
        parts.append(core_out)
    return np.ascontiguousarray(np.concatenate(parts, axis=-1))
